# revision 8
# baseline (speedup 1.0000x reference)
"""Two-layer GAT encoder on 8 TRN2 NeuronCores (Bass/Tile).

Strategy (graph-parallel, per sharding hint):
- Nodes are range-sharded across 8 cores (3750 per core, padded to 30
  blocks of 128). Edges (incl. self-loops) are bucketed by destination
  block and sorted by destination, padded to a uniform tile count T per
  block.
- Per layer, each core computes projected features for its own node
  shard: one matmul produces [xh | al_src | al_dst] at once, because the
  attention vectors are folded into extra weight columns
  (Ws[k,h] = sum_c W[k,h*C+c]*a[h,c]).  Rows [xh | al_src | pad] are
  packed fp16 (640 cols = 1280B, 256B-aligned for dma_gather) and
  AllGathered into a full 30000-row table in shared DRAM.
- Edge aggregation pulls rows of the table by source id with
  dma_gather (128 edges per partition-tile), builds a one-hot
  edge->dst_local matrix S on DVE (is_equal vs iota), computes per-edge
  logits e = al_src[gathered] + S''@al_dst (S'' = PE-transpose of S),
  w = exp(leaky_relu(e)), scales gathered features by w per head (ACT
  scale-copy), and scatter-adds via PE matmul S^T @ (w*G) with PSUM
  accumulation.  Softmax denominators come from a parallel matmul
  w^T @ S.  No max-subtraction is needed: logits are O(1) here, exp is
  safe in fp32.
- Layer 1 output h1 = relu(num/z) stays in SBUF; layer 2 repeats the
  pipeline with W2 and finishes with a head-mean.
"""

import numpy as np

import concourse.bacc as bacc
import concourse.mybir as mybir
import concourse.tile as tile
from concourse import bass_utils

N = 30000
IN = 256
HID = 128
H = 4
LAT = 128
NEG = 0.2
NCORES = 8
NS = N // NCORES          # 3750 nodes per core
NBLK = 30                 # dst blocks of 128 per core
NSP = NBLK * 128          # padded shard size 3840
D1 = H * HID              # 512
EXTC = D1 + 2 * H         # 520 = xh | al_src | al_dst
ROW = 640                 # table row elements (fp16) -> 1280B, 256B-aligned
F32 = mybir.dt.float32
F16 = mybir.dt.float16
I16 = mybir.dt.int16

_CACHE = {}


def _fold(W, a):
    # W [K, H*C] fp32, a [H, C] fp32 -> [K, H]
    K = W.shape[0]
    Hh, C = a.shape
    return np.einsum("khc,hc->kh", W.reshape(K, Hh, C).astype(np.float64),
                     a.astype(np.float64)).astype(np.float32)


def _build(T, add_b1, add_b2):
    nc = bacc.Bacc("TRN2", target_bir_lowering=False, debug=False,
                   num_devices=NCORES, num_swdge_queues=2)
    T8 = T * 8  # int16 idx cols per block (T*128/16)

    xT = nc.dram_tensor("xT", [IN, NSP], F16, kind="ExternalInput")
    w1e = nc.dram_tensor("w1e", [IN, EXTC], F16, kind="ExternalInput")
    w2e = nc.dram_tensor("w2e", [D1, EXTC], F16, kind="ExternalInput")
    srcidx = nc.dram_tensor("srcidx", [NBLK, 128, T8], I16, kind="ExternalInput")
    dstloc = nc.dram_tensor("dstloc", [NBLK, 128, T], F16, kind="ExternalInput")
    iota_in = nc.dram_tensor("iota_in", [128, T * 128], F16, kind="ExternalInput")
    ident_in = nc.dram_tensor("ident_in", [128, 128], F16, kind="ExternalInput")
    ident32_in = nc.dram_tensor("ident32_in", [128, 128], F32, kind="ExternalInput")
    if add_b1:
        b1rep = nc.dram_tensor("b1rep", [128, D1], F32, kind="ExternalInput")
    if add_b2:
        b2rep = nc.dram_tensor("b2rep", [128, LAT], F32, kind="ExternalInput")
    out_ext = nc.dram_tensor("out", [NS, LAT], F32, kind="ExternalOutput")

    with tile.TileContext(nc) as tc:
        with (
            tc.tile_pool(name="const", bufs=1) as const,
            tc.tile_pool(name="gpool", bufs=3) as gpool,
            tc.tile_pool(name="gs", bufs=3) as gs,
            tc.tile_pool(name="sp", bufs=3) as sp,
            tc.tile_pool(name="sm", bufs=3) as sm,
            tc.tile_pool(name="rows", bufs=3) as rows,
            tc.tile_pool(name="pbig", bufs=2, space="PSUM") as pbig,
            tc.tile_pool(name="ptr", bufs=2, space="PSUM") as ptr,
            tc.tile_pool(name="ps", bufs=2, space="PSUM") as psm,
            tc.tile_pool(name="pz", bufs=2, space="PSUM") as pzp,
            tc.tile_pool(name="dram", bufs=1, space="DRAM") as dram,
        ):
            # ---- constants / persistent SBUF
            iotaT = const.tile([128, T * 128], F16)
            nc.sync.dma_start(out=iotaT[:], in_=iota_in[:, :])
            ident = const.tile([128, 128], F16)
            nc.sync.dma_start(out=ident[:], in_=ident_in[:, :])
            ident32 = const.tile([128, 128], F32)
            nc.sync.dma_start(out=ident32[:], in_=ident32_in[:, :])
            w1sb = const.tile([128, 2, EXTC], F16)
            nc.sync.dma_start(out=w1sb[:], in_=w1e.ap().rearrange("(k p) n -> p k n", p=128))
            w2sb = const.tile([128, 4, EXTC], F16)
            nc.sync.dma_start(out=w2sb[:], in_=w2e.ap().rearrange("(k p) n -> p k n", p=128))
            xTsb = const.tile([128, 2, NSP], F16)
            nc.sync.dma_start(out=xTsb[:], in_=xT.ap().rearrange("(k p) n -> p k n", p=128))
            idxsb = const.tile([128, NBLK, T8], I16)
            nc.sync.dma_start(out=idxsb[:], in_=srcidx.ap().rearrange("b p s -> p b s"))
            dstsb = const.tile([128, NBLK, T], F16)
            nc.sync.dma_start(out=dstsb[:], in_=dstloc.ap().rearrange("b p t -> p b t"))
            if add_b1:
                b1sb = const.tile([128, D1], F32)
                nc.sync.dma_start(out=b1sb[:], in_=b1rep[:, :])
            if add_b2:
                b2sb = const.tile([128, LAT], F32)
                nc.sync.dma_start(out=b2sb[:], in_=b2rep[:, :])

            aldH1 = const.tile([128, NBLK, H], F16)
            aldH2 = const.tile([128, NBLK, H], F16)
            h1sb = const.tile([128, NBLK, D1], F16)

            bounce1 = dram.tile([NS, ROW], F16)
            table1 = dram.tile([N, ROW], F16, addr_space="Shared")
            bounce2 = dram.tile([NS, ROW], F16)
            table2 = dram.tile([N, ROW], F16, addr_space="Shared")

            Copy = mybir.ActivationFunctionType.Copy
            Relu = mybir.ActivationFunctionType.Relu
            Exp = mybir.ActivationFunctionType.Exp
            iseq = mybir.AluOpType.is_equal
            mult = mybir.AluOpType.mult
            add_ = mybir.AluOpType.add

            def nrows(j):
                return 128 if j < NBLK - 1 else NS - 128 * (NBLK - 1)

            # ---------- phase A: xh1 table shard ----------
            for j in range(NBLK):
                pa = pbig.tile([128, D1], F32, tag="pa")
                pb = psm.tile([128, 8], F32, tag="ps")
                for k in range(2):
                    lhs = xTsb[:, k, j * 128:(j + 1) * 128]
                    nc.tensor.matmul(pa[:], lhs, w1sb[:, k, 0:D1],
                                     start=(k == 0), stop=(k == 1))
                    nc.tensor.matmul(pb[:], lhs, w1sb[:, k, D1:EXTC],
                                     start=(k == 0), stop=(k == 1))
                row = rows.tile([128, ROW], F16, tag="row")
                nc.vector.memset(row[:, D1 + H:ROW], 0)
                nc.scalar.activation(row[:, 0:D1], pa[:], Copy)
                nc.vector.tensor_copy(row[:, D1:D1 + H], pb[:, 0:H])
                nc.vector.tensor_copy(aldH1[:, j, :], pb[:, H:2 * H])
                r = nrows(j)
                nc.sync.dma_start(out=bounce1[j * 128:j * 128 + r, :], in_=row[:r, :])

            nc.gpsimd.collective_compute(
                "AllGather", mybir.AluOpType.bypass,
                ins=[bounce1.opt()], outs=[table1.opt()],
                replica_groups=[list(range(NCORES))])

            # ---------- edge aggregation ----------
            def edge_phase(table, aldH, layer, after_block=None):
                for j in range(NBLK):
                    G = gpool.tile([128, T, ROW], F16, tag="G")
                    # SWDGE descriptor ring holds ~1024 descriptors; split the
                    # per-block gather into two chunks on alternating queues.
                    th = (T + 1) // 2
                    for qi, (t0, t1) in enumerate([(0, th), (th, T)]):
                        ni = (t1 - t0) * 128
                        nc.gpsimd.dma_gather(
                            G[:, t0:t1, :], table[:],
                            idxsb[:, j, t0 * 8:t1 * 8], ni, ni, ROW,
                            queue_num=(2 * j + qi) % 2)
                    # one-hot S for all T tiles in one DVE op
                    Sall = sp.tile([128, T, 128], F16, tag="S")
                    nc.vector.tensor_tensor(
                        out=Sall[:],
                        in0=dstsb[:, j, :].to_broadcast([128, T, 128]),
                        in1=iotaT[:].rearrange("p (t c) -> p t c", c=128),
                        op=iseq)
                    pa = pbig.tile([128, D1], F32, tag="pa")
                    zT = pzp.tile([4, 128], F32, tag="zT")
                    # batched transposes: groups of 4 tiles share one PSUM bank
                    S2g = {}
                    for g0 in range(0, T, 4):
                        g1 = min(g0 + 4, T)
                        pT = ptr.tile([128, 512], F16, tag="pt")
                        for u in range(g1 - g0):
                            nc.tensor.transpose(
                                pT[:, u * 128:(u + 1) * 128],
                                Sall[:, g0 + u, :], ident[:])
                        S2 = sp.tile([128, 512], F16, tag="S2")
                        nc.vector.tensor_copy(S2[:, 0:(g1 - g0) * 128],
                                              pT[:, 0:(g1 - g0) * 128])
                        S2g[g0] = S2
                    for t in range(T):
                        S2 = S2g[t // 4 * 4][:, (t % 4) * 128:(t % 4 + 1) * 128]
                        pald = psm.tile([128, 4], F32, tag="ps")
                        nc.tensor.matmul(pald[:], S2, aldH[:, j, :],
                                         start=True, stop=False)
                        nc.tensor.matmul(pald[:], ident[:], G[:, t, D1:D1 + H],
                                         start=False, stop=True)
                        esb = sm.tile([128, 4], F32, tag="esb")
                        nc.vector.tensor_copy(esb[:], pald[:])
                        wl = sm.tile([128, 4], F32, tag="wl")
                        nc.vector.scalar_tensor_tensor(
                            out=wl[:], in0=pald[:], scalar=NEG, in1=esb[:],
                            op0=mult, op1=mybir.AluOpType.max)
                        w16 = sm.tile([128, 4], F16, tag="w16")
                        nc.scalar.activation(w16[:], wl[:], Exp)
                        Gt = gs.tile([128, D1], F16, tag="Gt")
                        nc.vector.tensor_tensor(
                            out=Gt[:].rearrange("p (h c) -> p h c", c=128),
                            in0=G[:, t, 0:D1].rearrange("p (h c) -> p h c", c=128),
                            in1=w16[:].to_broadcast([128, H, 128]),
                            op=mult)
                        nc.tensor.matmul(pa[:], Sall[:, t, :], Gt[:],
                                         start=(t == 0), stop=(t == T - 1))
                        nc.tensor.matmul(zT[:], w16[:], Sall[:, t, :],
                                         start=(t == 0), stop=(t == T - 1))
                    # block finalize
                    zTs = sm.tile([4, 128], F32, tag="zTs")
                    nc.vector.tensor_copy(zTs[:], zT[:])
                    pzt = psm.tile([128, 4], F32, tag="ps")
                    nc.tensor.transpose(pzt[:], zTs[:], ident32[0:4, 0:4])
                    zeps = sm.tile([128, 4], F32, tag="zeps")
                    nc.vector.tensor_scalar_add(zeps[:], pzt[:], 1e-16)
                    rcp = sm.tile([128, 4], F32, tag="rcp")
                    nc.vector.reciprocal(rcp[:], zeps[:])
                    if layer == 1:
                        if add_b1:
                            tmp = rows.tile([128, D1], F32, tag="tmpb")
                            for h in range(H):
                                ch = slice(h * 128, (h + 1) * 128)
                                nc.scalar.activation(tmp[:, ch], pa[:, ch], Copy,
                                                     scale=rcp[:, h:h + 1])
                            nc.vector.tensor_add(tmp[:], tmp[:], b1sb[:])
                            nc.scalar.activation(h1sb[:, j, :], tmp[:], Relu)
                        else:
                            for h in range(H):
                                ch = slice(h * 128, (h + 1) * 128)
                                nc.scalar.activation(h1sb[:, j, ch], pa[:, ch],
                                                     Relu, scale=rcp[:, h:h + 1])
                    else:
                        rcp4 = sm.tile([128, 4], F32, tag="rcp4")
                        nc.vector.tensor_scalar_mul(rcp4[:], rcp[:], 0.25)
                        tmp2 = rows.tile([128, H, LAT], F32, tag="tmp2")
                        nc.vector.tensor_tensor(
                            out=tmp2[:],
                            in0=pa[:].rearrange("p (h c) -> p h c", c=128),
                            in1=rcp4[:].to_broadcast([128, H, LAT]),
                            op=mult)
                        o = rows.tile([128, LAT], F32, tag="o")
                        nc.vector.tensor_add(o[:], tmp2[:, 0, :], tmp2[:, 1, :])
                        nc.vector.tensor_add(o[:], o[:], tmp2[:, 2, :])
                        nc.vector.tensor_add(o[:], o[:], tmp2[:, 3, :])
                        if add_b2:
                            nc.vector.tensor_add(o[:], o[:], b2sb[:])
                        r = nrows(j)
                        nc.sync.dma_start(out=out_ext[j * 128:j * 128 + r, :],
                                          in_=o[:r, :])
                    if after_block is not None:
                        after_block(j)

            def emit_C(j):
                h1T = rows.tile([128, 4, 128], F16, tag="h1T")
                pT = ptr.tile([128, 512], F16, tag="pt")
                for k in range(4):
                    nc.tensor.transpose(pT[:, k * 128:(k + 1) * 128],
                                        h1sb[:, j, k * 128:(k + 1) * 128], ident[:])
                nc.vector.tensor_copy(h1T[:], pT[:].rearrange("p (k c) -> p k c", c=128))
                pa = pbig.tile([128, D1], F32, tag="pa")
                pb = psm.tile([128, 8], F32, tag="ps")
                for k in range(4):
                    nc.tensor.matmul(pa[:], h1T[:, k, :], w2sb[:, k, 0:D1],
                                     start=(k == 0), stop=(k == 3))
                    nc.tensor.matmul(pb[:], h1T[:, k, :], w2sb[:, k, D1:EXTC],
                                     start=(k == 0), stop=(k == 3))
                row = rows.tile([128, ROW], F16, tag="row")
                nc.vector.memset(row[:, D1 + H:ROW], 0)
                nc.scalar.activation(row[:, 0:D1], pa[:], Copy)
                nc.vector.tensor_copy(row[:, D1:D1 + H], pb[:, 0:H])
                nc.vector.tensor_copy(aldH2[:, j, :], pb[:, H:2 * H])
                r = nrows(j)
                nc.sync.dma_start(out=bounce2[j * 128:j * 128 + r, :], in_=row[:r, :])

            edge_phase(table1, aldH1, 1, after_block=emit_C)

            # phase C is interleaved into edge_phase(layer 1) above
            nc.gpsimd.collective_compute(
                "AllGather", mybir.AluOpType.bypass,
                ins=[bounce2.opt()], outs=[table2.opt()],
                replica_groups=[list(range(NCORES))])

            edge_phase(table2, aldH2, 2)

    nc.finalize()
    return nc


def _prep(inputs):
    x = np.asarray(inputs["x"], np.float32)
    ei = np.asarray(inputs["edge_index"], np.int64)
    W1 = np.asarray(inputs["W1"], np.float32)
    W2 = np.asarray(inputs["W2"], np.float32)

    src = np.concatenate([ei[0], np.arange(N, dtype=np.int64)])
    dst = np.concatenate([ei[1], np.arange(N, dtype=np.int64)])
    order = np.argsort(dst, kind="stable")
    ssrc = src[order]
    sdst = dst[order]

    # block boundaries: 240 global blocks of 128 dst ids (last block of each
    # core covers only 38 real ids: core boundary at 3750)
    blk_of_dst = (sdst // NS) * NBLK + (sdst % NS) // 128
    starts = np.searchsorted(blk_of_dst, np.arange(NCORES * NBLK))
    ends = np.searchsorted(blk_of_dst, np.arange(NCORES * NBLK), side="right")
    counts = ends - starts
    T = int(np.ceil(counts.max() / 128))

    srcidx_all = []
    dstloc_all = []
    for c in range(NCORES):
        si = np.zeros((NBLK, T * 128), np.int16)
        dl = np.full((NBLK, T * 128), 255.0, np.float16)
        for j in range(NBLK):
            g = c * NBLK + j
            s, e = starts[g], ends[g]
            cnt = e - s
            si[j, :cnt] = ssrc[s:e].astype(np.int16)
            dl[j, :cnt] = ((sdst[s:e] % NS) - j * 128).astype(np.float16)
        # wrap idx: slot i -> [i%16, i//16], replicated to 128 partitions
        w = si.reshape(NBLK, T * 8, 16).transpose(0, 2, 1)  # [NBLK,16,T8]
        w = np.tile(w, (1, 8, 1)).copy()                    # [NBLK,128,T8]
        srcidx_all.append(np.ascontiguousarray(w))
        # dstloc: [blk, p, t] = dst_in_block of slot t*128+p
        dstloc_all.append(np.ascontiguousarray(
            dl.reshape(NBLK, T, 128).transpose(0, 2, 1)))

    w1ext = np.concatenate(
        [W1, _fold(W1, np.asarray(inputs["a_src1"], np.float32)),
         _fold(W1, np.asarray(inputs["a_dst1"], np.float32))], axis=1
    ).astype(np.float16)
    w2ext = np.concatenate(
        [W2, _fold(W2, np.asarray(inputs["a_src2"], np.float32)),
         _fold(W2, np.asarray(inputs["a_dst2"], np.float32))], axis=1
    ).astype(np.float16)

    iota = np.tile(np.arange(128, dtype=np.float16), (128, T))
    ident = np.eye(128, dtype=np.float16)

    b1 = np.asarray(inputs["b1"], np.float32)
    b2 = np.asarray(inputs["b2"], np.float32)
    add_b1 = bool(np.any(b1))
    add_b2 = bool(np.any(b2))

    in_maps = []
    for c in range(NCORES):
        xs = np.zeros((IN, NSP), np.float16)
        xs[:, :NS] = x[c * NS:(c + 1) * NS].T.astype(np.float16)
        m = {
            "xT": xs,
            "w1e": w1ext,
            "w2e": w2ext,
            "srcidx": srcidx_all[c],
            "dstloc": dstloc_all[c],
            "iota_in": iota,
            "ident_in": ident,
            "ident32_in": ident.astype(np.float32),
        }
        if add_b1:
            m["b1rep"] = np.tile(b1, (128, 1)).astype(np.float32)
        if add_b2:
            m["b2rep"] = np.tile(b2, (128, 1)).astype(np.float32)
        in_maps.append(m)
    return T, add_b1, add_b2, in_maps


def _run(inputs, trace=False):
    T, add_b1, add_b2, in_maps = _prep(inputs)
    key = (T, add_b1, add_b2)
    if key not in _CACHE:
        _CACHE[key] = _build(T, add_b1, add_b2)
    nc = _CACHE[key]
    res = bass_utils.run_bass_kernel_spmd(
        nc, in_maps, core_ids=list(range(NCORES)), trace=trace)
    out = np.concatenate([res.results[c]["out"] for c in range(NCORES)], axis=0)
    return out.astype(np.float32), res


def kernel(**inputs):
    out, _ = _run(inputs, trace=False)
    return out
